# revision 1
# baseline (speedup 1.0000x reference)
import numpy as np

N = 8192
NFEAT = 512
NHID = 512
NCLASS = 64
NLAYERS = 8
LAMDA = 0.5
ALPHA = 0.1
NC = 8          # cores
RL = N // NC    # 1024 local rows per core
KT = N // 128   # 64 contraction tiles
MT = RL // 128  # 8 local row tiles
JT = NHID // 128  # 4 feature k-tiles for the W matmul


def _numpy_ref(x, adj, fc1_W, fc1_b, conv_Ws, fc2_W, fc2_b):
    n = adj.shape[0]
    A_hat = adj + np.eye(n, dtype=adj.dtype)
    dinv = 1.0 / np.sqrt(np.sum(A_hat, axis=0))
    P = dinv[:, None] * A_hat * dinv[None, :]
    H0 = np.maximum(x @ fc1_W + fc1_b, 0.0)
    H = H0
    for i in range(NLAYERS):
        beta = float(np.log(LAMDA / (i + 1) + 1.0))
        init_res = (1.0 - ALPHA) * (P @ H) + ALPHA * H0
        H = np.maximum((1.0 - beta) * init_res + beta * (init_res @ conv_Ws[i]), 0.0)
    logits = H @ fc2_W + fc2_b
    m = logits.max(axis=1, keepdims=True)
    lse = m + np.log(np.exp(logits - m).sum(axis=1, keepdims=True))
    return -(logits - lse)


def _build_nc():
    import concourse.bass as bass
    import concourse.mybir as mybir
    from concourse import tile

    dt = mybir.dt.float32
    nc = bass.Bass(target_bir_lowering=False, num_devices=NC)

    PT = nc.dram_tensor("PT", [N, RL], dt, kind="ExternalInput")        # 0.9*P[rows].T
    H0f = nc.dram_tensor("H0f", [N, NHID], dt, kind="ExternalInput")    # full H0
    H0a = nc.dram_tensor("H0a", [RL, NHID], dt, kind="ExternalInput")   # 0.1*H0 local rows
    Wt = nc.dram_tensor("Wt", [NLAYERS, NHID, NHID], dt, kind="ExternalInput")
    AI = nc.dram_tensor("AI", [128, 128], dt, kind="ExternalInput")     # 0.1*I... actually 1.0*I stationary for H0a
    Hout = nc.dram_tensor("Hout", [RL, NHID], dt, kind="ExternalOutput")

    h_loc = nc.dram_tensor("h_loc", [RL, NHID], dt)
    h_full = nc.dram_tensor("h_full", [N, NHID], dt)

    with tile.TileContext(nc) as tc:
        with (
            tc.tile_pool(name="res", bufs=1) as res,
            tc.tile_pool(name="wpool", bufs=2) as wpool,
            tc.tile_pool(name="ppool", bufs=4) as ppool,
            tc.tile_pool(name="mpool", bufs=2) as mpool,
            tc.tile_pool(name="tpool", bufs=2) as tpool,
            tc.tile_pool(name="npool", bufs=2) as npool,
            tc.tile_pool(name="psA", bufs=2, space="PSUM") as psA,
            tc.tile_pool(name="psT", bufs=2, space="PSUM") as psT,
            tc.tile_pool(name="psB", bufs=2, space="PSUM") as psB,
        ):
            Hsb = res.tile([128, KT, NHID], dt)       # full H resident: 128KB/part
            H0sb = res.tile([128, MT, NHID], dt)      # 0.1*H0 local rows
            ident = res.tile([128, 128], dt)

            nc.sync.dma_start(ident[:], AI[:, :])
            for m in range(MT):
                nc.sync.dma_start(H0sb[:, m, :], H0a[m * 128:(m + 1) * 128, :])
            for k in range(KT):
                nc.sync.dma_start(Hsb[:, k, :], H0f[k * 128:(k + 1) * 128, :])

            for l in range(NLAYERS):
                Wsb = wpool.tile([128, JT, NHID], dt, tag="w")
                for j in range(JT):
                    nc.sync.dma_start(Wsb[:, j, :], Wt[l, j * 128:(j + 1) * 128, :])

                for m in range(MT):
                    pa = psA.tile([128, NHID], dt, tag="pa")
                    for k in range(KT):
                        pt = ppool.tile([128, 128], dt, tag="pt")
                        nc.sync.dma_start(pt[:], PT[k * 128:(k + 1) * 128,
                                                    m * 128:(m + 1) * 128])
                        nc.tensor.matmul(pa[:], pt[:], Hsb[:, k, :],
                                         start=(k == 0), stop=False)
                    # += 1.0*I @ (0.1*H0_local[m])  -> adds alpha*H0 into psum
                    nc.tensor.matmul(pa[:], ident[:], H0sb[:, m, :],
                                     start=False, stop=True)

                    msb = mpool.tile([128, NHID], dt, tag="m")
                    nc.vector.tensor_copy(msb[:], pa[:])

                    pb = psB.tile([128, NHID], dt, tag="pb")
                    for j in range(JT):
                        ptr = psT.tile([128, 128], dt, tag="tr")
                        nc.tensor.transpose(ptr[:], msb[:, j * 128:(j + 1) * 128],
                                            ident[:])
                        mtj = tpool.tile([128, 128], dt, tag="mt")
                        nc.vector.tensor_copy(mtj[:], ptr[:])
                        nc.tensor.matmul(pb[:], mtj[:], Wsb[:, j, :],
                                         start=(j == 0), stop=(j == JT - 1))

                    hn = npool.tile([128, NHID], dt, tag="hn")
                    nc.scalar.activation(hn[:], pb[:],
                                         mybir.ActivationFunctionType.Relu,
                                         0.0, 1.0)
                    if l < NLAYERS - 1:
                        nc.sync.dma_start(h_loc[m * 128:(m + 1) * 128, :], hn[:])
                    else:
                        nc.sync.dma_start(Hout[m * 128:(m + 1) * 128, :], hn[:])

                if l < NLAYERS - 1:
                    nc.gpsimd.collective_compute(
                        "AllGather",
                        mybir.AluOpType.bypass,
                        replica_groups=[list(range(NC))],
                        ins=[h_loc[:, :]],
                        outs=[h_full[:, :]],
                    )
                    for k in range(KT):
                        nc.sync.dma_start(Hsb[:, k, :],
                                          h_full[k * 128:(k + 1) * 128, :])
    return nc


def kernel(**inputs):
    x = np.asarray(inputs["x"], np.float32)
    adj = np.asarray(inputs["adj"], np.float32)
    fc1_W = np.asarray(inputs["fc1_W"], np.float32)
    fc1_b = np.asarray(inputs["fc1_b"], np.float32)
    conv_Ws = np.asarray(inputs["conv_Ws"], np.float32)
    fc2_W = np.asarray(inputs["fc2_W"], np.float32)
    fc2_b = np.asarray(inputs["fc2_b"], np.float32)
    try:
        A_hat = adj + np.eye(N, dtype=np.float32)
        dinv = (1.0 / np.sqrt(A_hat.sum(axis=0))).astype(np.float32)
        P = dinv[:, None] * A_hat * dinv[None, :]
        H0 = np.maximum(x @ fc1_W + fc1_b, 0.0).astype(np.float32)

        betas = [float(np.log(LAMDA / (i + 1) + 1.0)) for i in range(NLAYERS)]
        I512 = np.eye(NHID, dtype=np.float32)
        Wt = np.stack([(1.0 - betas[i]) * I512 + betas[i] * conv_Ws[i]
                       for i in range(NLAYERS)]).astype(np.float32)
        AI = np.eye(128, dtype=np.float32)
        H0a_full = (ALPHA * H0).astype(np.float32)
        Psc = ((1.0 - ALPHA) * P).astype(np.float32)

        in_maps = []
        for c in range(NC):
            r0, r1 = c * RL, (c + 1) * RL
            in_maps.append({
                "PT": np.ascontiguousarray(Psc[r0:r1, :].T),
                "H0f": H0,
                "H0a": np.ascontiguousarray(H0a_full[r0:r1, :]),
                "Wt": Wt,
                "AI": AI,
            })

        from concourse.bass_utils import run_bass_kernel_spmd
        nc = _build_nc()
        res = run_bass_kernel_spmd(nc, in_maps, core_ids=list(range(NC)))
        outs = res.results
        H8 = np.concatenate([np.asarray(outs[c]["Hout"]) for c in range(NC)], axis=0)

        logits = H8 @ fc2_W + fc2_b
        m = logits.max(axis=1, keepdims=True)
        lse = m + np.log(np.exp(logits - m).sum(axis=1, keepdims=True))
        return (-(logits - lse)).astype(np.float32)
    except Exception:
        import traceback
        traceback.print_exc()
        return _numpy_ref(x, adj, fc1_W, fc1_b, conv_Ws, fc2_W, fc2_b)



# revision 4
# speedup vs baseline: 2.5352x; 2.5352x over previous
import os
import numpy as np

# nn_GCNII: H0 = relu(x@W1+b1); 8 layers of
#   M = 0.9*(P@H) + 0.1*H0 ; H = relu(M @ ((1-b)I + b*W_l)) ; out = -log_softmax(H@W2+b2)
# with P = D^-1/2 (A+I) D^-1/2 fixed.
#
# Device strategy (8 cores, row-sharded):
#  * Work in "tilde" space G = dinv * H. Then P@H = dinv * ((A+I) @ G) and the
#    per-layer update folds into per-partition scales:
#      pa  = (A+I)@G + I@B0c          (B0c = H0/(9*dinv), added via identity matmul)
#      Mt  = c2 * pa                  (c2 = 0.9*dinv^2; last layer uses c1 = 0.9*dinv)
#      G'  = relu(Mt @ Wt_l)          (Wt_l = (1-b)I + b*W_l)
#  * (A+I) entries are {0,1,2} -> exact in fp8; the 8MB/core slice stays SBUF-resident.
#  * Activations move in bf16 (1 cycle/row on PE vs 4 for fp32).
#  * Per-layer bf16 AllGather of G, chunked to overlap with compute.

N = 8192
NC = 8
RL = N // NC          # 1024 local rows
F = 512               # NFEAT == NHID
NCLASS = 64
NLAYERS = 8
LAMDA = 0.5
ALPHA = 0.1
NCH = 2               # allgather chunks per step

LAST_RESULTS = None
USED_FALLBACK = False


def _numpy_ref(x, adj, fc1_W, fc1_b, conv_Ws, fc2_W, fc2_b):
    n = adj.shape[0]
    A_hat = adj + np.eye(n, dtype=adj.dtype)
    dinv = 1.0 / np.sqrt(np.sum(A_hat, axis=0))
    P = dinv[:, None] * A_hat * dinv[None, :]
    H0 = np.maximum(x @ fc1_W + fc1_b, 0.0)
    H = H0
    for i in range(NLAYERS):
        beta = float(np.log(LAMDA / (i + 1) + 1.0))
        init_res = (1.0 - ALPHA) * (P @ H) + ALPHA * H0
        H = np.maximum((1.0 - beta) * init_res + beta * (init_res @ conv_Ws[i]), 0.0)
    logits = H @ fc2_W + fc2_b
    m = logits.max(axis=1, keepdims=True)
    lse = m + np.log(np.exp(logits - m).sum(axis=1, keepdims=True))
    return -(logits - lse)


def build_nc(n=N, nch=NCH):
    import concourse.bass as bass
    import concourse.mybir as mybir
    from concourse import tile

    f32 = mybir.dt.float32
    bf16 = mybir.dt.bfloat16
    f8 = mybir.dt.float8e4

    rl = n // NC
    mt = rl // 128        # local row tiles
    kt = n // 128         # contraction tiles
    jt = F // 128         # feature tiles
    mtch = mt // nch      # m-tiles per AG chunk
    ktch = kt // nch      # k-tiles per chunk
    csl = rl // nch       # local rows per chunk

    nc = bass.Bass(target_bir_lowering=False, num_devices=NC)

    AT = nc.dram_tensor("AT", [n, rl], f8, kind="ExternalInput")
    XT = nc.dram_tensor("XT", [F, rl], bf16, kind="ExternalInput")
    W1 = nc.dram_tensor("W1", [F, F], bf16, kind="ExternalInput")
    B1 = nc.dram_tensor("B1", [1, F], bf16, kind="ExternalInput")
    WT = nc.dram_tensor("WT", [NLAYERS, F, F], bf16, kind="ExternalInput")
    W2 = nc.dram_tensor("W2", [F, NCLASS], bf16, kind="ExternalInput")
    B2 = nc.dram_tensor("B2", [1, NCLASS], bf16, kind="ExternalInput")
    IDT = nc.dram_tensor("IDT", [128, 128], bf16, kind="ExternalInput")
    ONE = nc.dram_tensor("ONE", [1, 128], bf16, kind="ExternalInput")
    SC = nc.dram_tensor("SC", [128, mt, 4], f32, kind="ExternalInput")
    OUT = nc.dram_tensor("OUT", [rl, NCLASS], f32, kind="ExternalOutput")

    gin = [[nc.dram_tensor(f"gin{s}_{q}", [csl, F], bf16)
            for q in range(nch)] for s in range(NLAYERS)]
    gf = [[nc.dram_tensor(f"gf{s}_{q}", [NC * csl, F], bf16, addr_space="Shared")
           for q in range(nch)] for s in range(NLAYERS)]

    AF = mybir.ActivationFunctionType
    ALU = mybir.AluOpType
    groups = [list(range(NC))]

    with tile.TileContext(nc) as tc:
        with (
            tc.tile_pool(name="res", bufs=1) as res,
            tc.tile_pool(name="gres", bufs=1) as gres,
            tc.tile_pool(name="gchp", bufs=2) as gchp,
            tc.tile_pool(name="mpool", bufs=3) as mpool,
            tc.tile_pool(name="tpool", bufs=4) as tpool,
            tc.tile_pool(name="spool", bufs=2) as spool,
            tc.tile_pool(name="opool", bufs=2) as opool,
            tc.tile_pool(name="psA", bufs=2, space="PSUM") as psA,
            tc.tile_pool(name="psB", bufs=2, space="PSUM") as psB,
            tc.tile_pool(name="psT", bufs=2, space="PSUM") as psT,
        ):
            ATsb = res.tile([128, kt, rl], f8)
            WTsb = res.tile([128, NLAYERS, jt, F], bf16)
            B0csb = res.tile([128, mt, F], bf16)
            XTsb = res.tile([128, jt, rl], bf16)
            HFsb = res.tile([128, mt, F], bf16)
            W1sb = res.tile([128, jt, F], bf16)
            W2sb = res.tile([128, jt, NCLASS], bf16)
            IDTsb = res.tile([128, 128], bf16)
            ONEsb = res.tile([1, 128], bf16)
            B1sb = res.tile([1, F], bf16)
            B2sb = res.tile([1, NCLASS], bf16)
            SCsb = res.tile([128, mt, 4], f32)

            Gsb = [gres.tile([128, ktch, F], bf16, tag=f"g{q}", name=f"Gsb{q}")
                   for q in range(nch)]

            nc.sync.dma_start(IDTsb[:], IDT[:, :])
            nc.sync.dma_start(ONEsb[:], ONE[:, :])
            nc.sync.dma_start(B1sb[:], B1[:, :])
            nc.sync.dma_start(B2sb[:], B2[:, :])
            nc.sync.dma_start(SCsb[:], SC[:, :, :])
            nc.sync.dma_start(W1sb[:], W1[:, :].rearrange("(j p) f -> p j f", p=128))
            nc.sync.dma_start(W2sb[:], W2[:, :].rearrange("(j p) f -> p j f", p=128))
            nc.sync.dma_start(XTsb[:], XT[:, :].rearrange("(j p) m -> p j m", p=128))
            for l in range(NLAYERS):
                nc.sync.dma_start(WTsb[:, l, :, :],
                                  WT[l, :, :].rearrange("(j p) f -> p j f", p=128))
            nblk = 8 if kt % 8 == 0 else 1
            for b in range(0, kt, nblk):
                nc.sync.dma_start(
                    ATsb[:, b:b + nblk, :],
                    AT[b * 128:(b + nblk) * 128, :].rearrange("(k p) m -> p k m", p=128))

            # ---- fc1: H0 = relu(x@W1 + b1); emit G0 = dinv*H0, B0c = H0/(9*dinv)
            with nc.named_scope("fc1"):
                for q in range(nch):
                    gch = gchp.tile([128, mtch, F], bf16, tag="gch")
                    for mi in range(mtch):
                        m = q * mtch + mi
                        p0 = psA.tile([128, F], f32, tag="pa")
                        nc.tensor.matmul(p0[:], ONEsb[:], B1sb[:], start=True, stop=False)
                        for j in range(jt):
                            nc.tensor.matmul(p0[:], XTsb[:, j, m * 128:(m + 1) * 128],
                                             W1sb[:, j, :], start=False, stop=(j == jt - 1))
                        nc.scalar.activation(gch[:, mi, :], p0[:], AF.Relu,
                                             0.0, SCsb[:, m, 0:1])
                        nc.scalar.activation(B0csb[:, m, :], p0[:], AF.Relu,
                                             0.0, SCsb[:, m, 1:2])
                    nc.sync.dma_start(
                        gin[0][q][:, :].rearrange("(mi p) f -> p mi f", p=128), gch[:])
                    nc.gpsimd.collective_compute(
                        "AllGather", ALU.bypass, replica_groups=groups,
                        ins=[gin[0][q][:, :]], outs=[gf[0][q][:, :]])

            # ---- GCNII layers
            for l in range(NLAYERS):
                with nc.named_scope(f"L{l}"):
                    for q in range(nch):
                        nc.sync.dma_start(
                            Gsb[q][:],
                            gf[l][q][:, :].rearrange("(k p) f -> p k f", p=128))
                    gch = None
                    for m in range(mt):
                        q, mi = divmod(m, mtch)
                        pa = psA.tile([128, F], f32, tag="pa")
                        for cq in range(nch):
                            for kk in range(ktch):
                                ktile = cq * ktch + kk
                                nc.tensor.matmul(
                                    pa[:], ATsb[:, ktile, m * 128:(m + 1) * 128],
                                    Gsb[cq][:, kk, :], start=(ktile == 0), stop=False)
                        nc.tensor.matmul(pa[:], IDTsb[:], B0csb[:, m, :],
                                         start=False, stop=True)
                        msb = mpool.tile([128, F], bf16, tag="m")
                        sidx = 2 if l < NLAYERS - 1 else 3
                        nc.scalar.activation(msb[:], pa[:], AF.Copy,
                                             0.0, SCsb[:, m, sidx:sidx + 1])
                        pb = psB.tile([128, F], f32, tag="pb")
                        for j in range(jt):
                            ptr = psT.tile([128, 128], bf16, tag="tr")
                            nc.tensor.transpose(ptr[:], msb[:, j * 128:(j + 1) * 128],
                                                IDTsb[:])
                            mtj = tpool.tile([128, 128], bf16, tag="mt")
                            nc.vector.tensor_copy(mtj[:], ptr[:])
                            nc.tensor.matmul(pb[:], mtj[:], WTsb[:, l, j, :],
                                             start=(j == 0), stop=(j == jt - 1))
                        if l < NLAYERS - 1:
                            if mi == 0:
                                gch = gchp.tile([128, mtch, F], bf16, tag="gch")
                            nc.scalar.activation(gch[:, mi, :], pb[:], AF.Relu)
                            if mi == mtch - 1:
                                nc.sync.dma_start(
                                    gin[l + 1][q][:, :].rearrange(
                                        "(mi p) f -> p mi f", p=128), gch[:])
                                nc.gpsimd.collective_compute(
                                    "AllGather", ALU.bypass, replica_groups=groups,
                                    ins=[gin[l + 1][q][:, :]], outs=[gf[l + 1][q][:, :]])
                        else:
                            nc.scalar.activation(HFsb[:, m, :], pb[:], AF.Relu)

            # ---- fc2 + -log_softmax
            with nc.named_scope("fc2"):
                for m in range(mt):
                    pl = psB.tile([128, NCLASS], f32, tag="pb")
                    nc.tensor.matmul(pl[:], ONEsb[:], B2sb[:], start=True, stop=False)
                    for j in range(jt):
                        ptr = psT.tile([128, 128], bf16, tag="tr")
                        nc.tensor.transpose(ptr[:], HFsb[:, m, j * 128:(j + 1) * 128],
                                            IDTsb[:])
                        htj = tpool.tile([128, 128], bf16, tag="mt")
                        nc.vector.tensor_copy(htj[:], ptr[:])
                        nc.tensor.matmul(pl[:], htj[:], W2sb[:, j, :],
                                         start=False, stop=(j == jt - 1))
                    nmax = spool.tile([128, 1], f32, tag="nm")
                    nc.vector.tensor_reduce(nmax[:], pl[:], op=ALU.max,
                                            axis=mybir.AxisListType.X, negate=True)
                    esb = opool.tile([128, NCLASS], f32, tag="e")
                    ssum = spool.tile([128, 1], f32, tag="ss")
                    nc.scalar.activation(esb[:], pl[:], AF.Exp, nmax[:, 0:1], 1.0,
                                         accum_out=ssum[:])
                    lse = spool.tile([128, 1], f32, tag="ls")
                    nc.scalar.activation(lse[:], ssum[:], AF.Ln)
                    s2 = spool.tile([128, 1], f32, tag="s2")
                    nc.vector.tensor_tensor(s2[:], lse[:], nmax[:], ALU.subtract)
                    osb = opool.tile([128, NCLASS], f32, tag="o")
                    nc.vector.tensor_scalar(osb[:], pl[:], s2[:, 0:1], -1.0,
                                            op0=ALU.subtract, op1=ALU.mult)
                    nc.sync.dma_start(OUT[m * 128:(m + 1) * 128, :], osb[:])
    return nc


def host_prep(x, adj, fc1_W, fc1_b, conv_Ws, fc2_W, fc2_b, n=N, nch=NCH):
    import ml_dtypes
    bf16 = ml_dtypes.bfloat16
    f8 = ml_dtypes.float8_e4m3

    rl = n // NC
    mt = rl // 128
    csl = rl // nch

    colsum = adj.sum(axis=0, dtype=np.float32) + 1.0
    dinv = (1.0 / np.sqrt(colsum)).astype(np.float32)

    # AT row permutation: AG chunk q concatenates cores' chunk-q slices, so
    # gathered G row order is (q, c, j); A^T rows must match.
    idx = np.arange(n).reshape(NC, nch, csl).transpose(1, 0, 2).ravel()

    betas = [float(np.log(LAMDA / (i + 1) + 1.0)) for i in range(NLAYERS)]
    I512 = np.eye(F, dtype=np.float32)
    WTh = np.stack([(1.0 - betas[i]) * I512 + betas[i] * conv_Ws[i]
                    for i in range(NLAYERS)]).astype(bf16)
    W1h = fc1_W.astype(bf16)
    B1h = fc1_b.reshape(1, F).astype(bf16)
    W2h = fc2_W.astype(bf16)
    B2h = fc2_b.reshape(1, NCLASS).astype(bf16)
    IDTh = np.eye(128, dtype=bf16)
    ONEh = np.ones((1, 128), dtype=bf16)

    md = np.arange(rl)
    pnew_diag = (md // csl) * (NC * csl) + (md % csl)  # + c*csl per core below

    in_maps = []
    for c in range(NC):
        r0 = c * rl
        ATf = adj[r0:r0 + rl, :].T[idx]          # [n, rl] permuted-row A^T slice
        ATf[pnew_diag + c * csl, md] += 1.0      # + I at permuted positions
        dloc = dinv[r0:r0 + rl]
        sc = np.empty((128, mt, 4), np.float32)
        sc[:, :, 0] = dloc.reshape(mt, 128).T
        sc[:, :, 1] = (1.0 / (9.0 * dloc)).reshape(mt, 128).T
        sc[:, :, 2] = (0.9 * dloc * dloc).reshape(mt, 128).T
        sc[:, :, 3] = (0.9 * dloc).reshape(mt, 128).T
        in_maps.append({
            "AT": ATf.astype(f8),
            "XT": np.ascontiguousarray(x[r0:r0 + rl, :].T).astype(bf16),
            "W1": W1h, "B1": B1h, "WT": WTh, "W2": W2h, "B2": B2h,
            "IDT": IDTh, "ONE": ONEh, "SC": sc,
        })
    return in_maps


def kernel(**inputs):
    global LAST_RESULTS, USED_FALLBACK
    x = np.asarray(inputs["x"], np.float32)
    adj = np.asarray(inputs["adj"], np.float32)
    fc1_W = np.asarray(inputs["fc1_W"], np.float32)
    fc1_b = np.asarray(inputs["fc1_b"], np.float32)
    conv_Ws = np.asarray(inputs["conv_Ws"], np.float32)
    fc2_W = np.asarray(inputs["fc2_W"], np.float32)
    fc2_b = np.asarray(inputs["fc2_b"], np.float32)
    try:
        in_maps = host_prep(x, adj, fc1_W, fc1_b, conv_Ws, fc2_W, fc2_b)
        from concourse.bass_utils import run_bass_kernel_spmd
        nc = build_nc()
        res = run_bass_kernel_spmd(nc, in_maps, core_ids=list(range(NC)))
        LAST_RESULTS = res
        out = np.concatenate([np.asarray(res.results[c]["OUT"]) for c in range(NC)],
                             axis=0).astype(np.float32)
        return out
    except Exception:
        import traceback
        traceback.print_exc()
        USED_FALLBACK = True
        return _numpy_ref(x, adj, fc1_W, fc1_b, conv_Ws, fc2_W, fc2_b)


# revision 7
# speedup vs baseline: 11750.4408x; 4634.8604x over previous
import os
import numpy as np

# nn_GCNII: H0 = relu(x@W1+b1); 8 layers of
#   M = 0.9*(P@H) + 0.1*H0 ; H = relu(M @ ((1-b)I + b*W_l)) ; out = -log_softmax(H@W2+b2)
# with P = D^-1/2 (A+I) D^-1/2 fixed.
#
# Device strategy (8 cores, row-sharded):
#  * Work in "tilde" space G = dinv * H. Then P@H = dinv * ((A+I) @ G) and the
#    per-layer update folds into per-partition scales:
#      pa  = (A+I)@G + I@B0c          (B0c = H0/(9*dinv), added via identity matmul)
#      Mt  = c2 * pa                  (c2 = 0.9*dinv^2; last layer uses c1 = 0.9*dinv)
#      G'  = relu(Mt @ Wt_l)          (Wt_l = (1-b)I + b*W_l)
#  * (A+I) entries are {0,1,2} -> exact in fp8; the 8MB/core slice stays SBUF-resident.
#  * Activations move in bf16 (1 cycle/row on PE vs 4 for fp32).
#  * Per-layer bf16 AllGather of G, chunked to overlap with compute.

N = 8192
NC = 8
RL = N // NC          # 1024 local rows
F = 512               # NFEAT == NHID
NCLASS = 64
NLAYERS = 8
LAMDA = 0.5
ALPHA = 0.1
NCH = 2               # allgather chunks per step

LAST_RESULTS = None
USED_FALLBACK = False


def _numpy_ref(x, adj, fc1_W, fc1_b, conv_Ws, fc2_W, fc2_b):
    n = adj.shape[0]
    A_hat = adj + np.eye(n, dtype=adj.dtype)
    dinv = 1.0 / np.sqrt(np.sum(A_hat, axis=0))
    P = dinv[:, None] * A_hat * dinv[None, :]
    H0 = np.maximum(x @ fc1_W + fc1_b, 0.0)
    H = H0
    for i in range(NLAYERS):
        beta = float(np.log(LAMDA / (i + 1) + 1.0))
        init_res = (1.0 - ALPHA) * (P @ H) + ALPHA * H0
        H = np.maximum((1.0 - beta) * init_res + beta * (init_res @ conv_Ws[i]), 0.0)
    logits = H @ fc2_W + fc2_b
    m = logits.max(axis=1, keepdims=True)
    lse = m + np.log(np.exp(logits - m).sum(axis=1, keepdims=True))
    return -(logits - lse)


def build_nc(n=N, nch=NCH):
    import concourse.bacc as bacc
    import concourse.mybir as mybir
    from concourse import tile

    f32 = mybir.dt.float32
    bf16 = mybir.dt.bfloat16
    f8 = mybir.dt.float8e4

    rl = n // NC
    mt = rl // 128        # local row tiles
    kt = n // 128         # contraction tiles
    jt = F // 128         # feature tiles
    mtch = mt // nch      # m-tiles per AG chunk
    ktch = kt // nch      # k-tiles per chunk
    csl = rl // nch       # local rows per chunk

    # Bacc (not Bass): its finalize() runs generate_event_semaphores, which
    # splits multi-semaphore waits — walrus codegen allows only 1 sync wait
    # per instruction on TRN2 ("Too many sync wait commands" otherwise).
    nc = bacc.Bacc(target_bir_lowering=False, num_devices=NC)

    AT = nc.dram_tensor("AT", [n, rl], f8, kind="ExternalInput")
    XT = nc.dram_tensor("XT", [F, rl], bf16, kind="ExternalInput")
    W1 = nc.dram_tensor("W1", [F, F], bf16, kind="ExternalInput")
    B1 = nc.dram_tensor("B1", [1, F], bf16, kind="ExternalInput")
    WT = nc.dram_tensor("WT", [NLAYERS, F, F], bf16, kind="ExternalInput")
    W2 = nc.dram_tensor("W2", [F, NCLASS], bf16, kind="ExternalInput")
    B2 = nc.dram_tensor("B2", [1, NCLASS], bf16, kind="ExternalInput")
    IDT = nc.dram_tensor("IDT", [128, 128], bf16, kind="ExternalInput")
    ONE = nc.dram_tensor("ONE", [1, 128], bf16, kind="ExternalInput")
    SC = nc.dram_tensor("SC", [128, mt, 4], f32, kind="ExternalInput")
    OUT = nc.dram_tensor("OUT", [rl, NCLASS], f32, kind="ExternalOutput")

    gin = [[nc.dram_tensor(f"gin{s}_{q}", [csl, F], bf16)
            for q in range(nch)] for s in range(NLAYERS)]
    gf = [[nc.dram_tensor(f"gf{s}_{q}", [NC * csl, F], bf16, addr_space="Shared")
           for q in range(nch)] for s in range(NLAYERS)]

    AF = mybir.ActivationFunctionType
    ALU = mybir.AluOpType
    groups = [list(range(NC))]

    with tile.TileContext(nc) as tc:
        with (
            tc.tile_pool(name="res", bufs=1) as res,
            tc.tile_pool(name="gres", bufs=1) as gres,
            tc.tile_pool(name="gchp", bufs=2) as gchp,
            tc.tile_pool(name="mpool", bufs=3) as mpool,
            tc.tile_pool(name="tpool", bufs=4) as tpool,
            tc.tile_pool(name="spool", bufs=2) as spool,
            tc.tile_pool(name="opool", bufs=2) as opool,
            tc.tile_pool(name="psA", bufs=2, space="PSUM") as psA,
            tc.tile_pool(name="psB", bufs=2, space="PSUM") as psB,
            tc.tile_pool(name="psT", bufs=2, space="PSUM") as psT,
        ):
            ATsb = res.tile([128, kt, rl], f8)
            WTsb = res.tile([128, NLAYERS, jt, F], bf16)
            B0csb = res.tile([128, mt, F], bf16)
            XTsb = res.tile([128, jt, rl], bf16)
            HFsb = res.tile([128, mt, F], bf16)
            W1sb = res.tile([128, jt, F], bf16)
            W2sb = res.tile([128, jt, NCLASS], bf16)
            IDTsb = res.tile([128, 128], bf16)
            ONEsb = res.tile([1, 128], bf16)
            B1sb = res.tile([1, F], bf16)
            B2sb = res.tile([1, NCLASS], bf16)
            SCsb = res.tile([128, mt, 4], f32)

            Gsb = [gres.tile([128, ktch, F], bf16, tag=f"g{q}", name=f"Gsb{q}")
                   for q in range(nch)]

            nc.sync.dma_start(IDTsb[:], IDT[:, :])
            nc.sync.dma_start(ONEsb[:], ONE[:, :])
            nc.sync.dma_start(B1sb[:], B1[:, :])
            nc.sync.dma_start(B2sb[:], B2[:, :])
            nc.sync.dma_start(SCsb[:], SC[:, :, :])
            nc.sync.dma_start(W1sb[:], W1[:, :].rearrange("(j p) f -> p j f", p=128))
            nc.sync.dma_start(W2sb[:], W2[:, :].rearrange("(j p) f -> p j f", p=128))
            nc.sync.dma_start(XTsb[:], XT[:, :].rearrange("(j p) m -> p j m", p=128))
            for l in range(NLAYERS):
                nc.sync.dma_start(WTsb[:, l, :, :],
                                  WT[l, :, :].rearrange("(j p) f -> p j f", p=128))
            nblk = 8 if kt % 8 == 0 else 1
            for b in range(0, kt, nblk):
                nc.sync.dma_start(
                    ATsb[:, b:b + nblk, :],
                    AT[b * 128:(b + nblk) * 128, :].rearrange("(k p) m -> p k m", p=128))

            # ---- fc1: H0 = relu(x@W1 + b1); emit G0 = dinv*H0, B0c = H0/(9*dinv)
            with nc.named_scope("fc1"):
                for q in range(nch):
                    gch = gchp.tile([128, mtch, F], bf16, tag="gch")
                    for mi in range(mtch):
                        m = q * mtch + mi
                        p0 = psA.tile([128, F], f32, tag="pa")
                        nc.tensor.matmul(p0[:], ONEsb[:], B1sb[:], start=True, stop=False)
                        for j in range(jt):
                            nc.tensor.matmul(p0[:], XTsb[:, j, m * 128:(m + 1) * 128],
                                             W1sb[:, j, :], start=False, stop=(j == jt - 1))
                        nc.scalar.activation(gch[:, mi, :], p0[:], AF.Relu,
                                             0.0, SCsb[:, m, 0:1])
                        nc.scalar.activation(B0csb[:, m, :], p0[:], AF.Relu,
                                             0.0, SCsb[:, m, 1:2])
                    nc.sync.dma_start(
                        gin[0][q][:, :].rearrange("(mi p) f -> p mi f", p=128), gch[:])
                    nc.gpsimd.collective_compute(
                        "AllGather", ALU.bypass, replica_groups=groups,
                        ins=[gin[0][q][:, :]], outs=[gf[0][q][:, :]])

            # ---- GCNII layers
            for l in range(NLAYERS):
                with nc.named_scope(f"L{l}"):
                    for q in range(nch):
                        nc.sync.dma_start(
                            Gsb[q][:],
                            gf[l][q][:, :].rearrange("(k p) f -> p k f", p=128))
                    gch = None
                    for m in range(mt):
                        q, mi = divmod(m, mtch)
                        pa = psA.tile([128, F], f32, tag="pa")
                        for cq in range(nch):
                            for kk in range(ktch):
                                ktile = cq * ktch + kk
                                nc.tensor.matmul(
                                    pa[:], ATsb[:, ktile, m * 128:(m + 1) * 128],
                                    Gsb[cq][:, kk, :], start=(ktile == 0), stop=False)
                        nc.tensor.matmul(pa[:], IDTsb[:], B0csb[:, m, :],
                                         start=False, stop=True)
                        msb = mpool.tile([128, F], bf16, tag="m")
                        sidx = 2 if l < NLAYERS - 1 else 3
                        nc.scalar.activation(msb[:], pa[:], AF.Copy,
                                             0.0, SCsb[:, m, sidx:sidx + 1])
                        pb = psB.tile([128, F], f32, tag="pb")
                        for j in range(jt):
                            ptr = psT.tile([128, 128], bf16, tag="tr")
                            nc.tensor.transpose(ptr[:], msb[:, j * 128:(j + 1) * 128],
                                                IDTsb[:])
                            mtj = tpool.tile([128, 128], bf16, tag="mt")
                            nc.vector.tensor_copy(mtj[:], ptr[:])
                            nc.tensor.matmul(pb[:], mtj[:], WTsb[:, l, j, :],
                                             start=(j == 0), stop=(j == jt - 1))
                        if l < NLAYERS - 1:
                            if mi == 0:
                                gch = gchp.tile([128, mtch, F], bf16, tag="gch")
                            nc.scalar.activation(gch[:, mi, :], pb[:], AF.Relu)
                            if mi == mtch - 1:
                                nc.sync.dma_start(
                                    gin[l + 1][q][:, :].rearrange(
                                        "(mi p) f -> p mi f", p=128), gch[:])
                                nc.gpsimd.collective_compute(
                                    "AllGather", ALU.bypass, replica_groups=groups,
                                    ins=[gin[l + 1][q][:, :]], outs=[gf[l + 1][q][:, :]])
                        else:
                            nc.scalar.activation(HFsb[:, m, :], pb[:], AF.Relu)

            # ---- fc2 + -log_softmax
            with nc.named_scope("fc2"):
                for m in range(mt):
                    pl = psB.tile([128, NCLASS], f32, tag="pb")
                    nc.tensor.matmul(pl[:], ONEsb[:], B2sb[:], start=True, stop=False)
                    for j in range(jt):
                        ptr = psT.tile([128, 128], bf16, tag="tr")
                        nc.tensor.transpose(ptr[:], HFsb[:, m, j * 128:(j + 1) * 128],
                                            IDTsb[:])
                        htj = tpool.tile([128, 128], bf16, tag="mt")
                        nc.vector.tensor_copy(htj[:], ptr[:])
                        nc.tensor.matmul(pl[:], htj[:], W2sb[:, j, :],
                                         start=False, stop=(j == jt - 1))
                    nmax = spool.tile([128, 1], f32, tag="nm")
                    nc.vector.tensor_reduce(nmax[:], pl[:], op=ALU.max,
                                            axis=mybir.AxisListType.X, negate=True)
                    esb = opool.tile([128, NCLASS], f32, tag="e")
                    ssum = spool.tile([128, 1], f32, tag="ss")
                    nc.scalar.activation(esb[:], pl[:], AF.Exp, nmax[:, 0:1], 1.0,
                                         accum_out=ssum[:])
                    lse = spool.tile([128, 1], f32, tag="ls")
                    nc.scalar.activation(lse[:], ssum[:], AF.Ln)
                    s2 = spool.tile([128, 1], f32, tag="s2")
                    nc.vector.tensor_tensor(s2[:], lse[:], nmax[:], ALU.subtract)
                    osb = opool.tile([128, NCLASS], f32, tag="o")
                    nc.vector.tensor_scalar(osb[:], pl[:], s2[:, 0:1], -1.0,
                                            op0=ALU.subtract, op1=ALU.mult)
                    nc.sync.dma_start(OUT[m * 128:(m + 1) * 128, :], osb[:])
    nc.finalize()
    return nc


def host_prep(x, adj, fc1_W, fc1_b, conv_Ws, fc2_W, fc2_b, n=N, nch=NCH):
    import ml_dtypes
    bf16 = ml_dtypes.bfloat16
    f8 = ml_dtypes.float8_e4m3

    rl = n // NC
    mt = rl // 128
    csl = rl // nch

    colsum = adj.sum(axis=0, dtype=np.float32) + 1.0
    dinv = (1.0 / np.sqrt(colsum)).astype(np.float32)

    # AT row permutation: AG chunk q concatenates cores' chunk-q slices, so
    # gathered G row order is (q, c, j); A^T rows must match.
    idx = np.arange(n).reshape(NC, nch, csl).transpose(1, 0, 2).ravel()

    betas = [float(np.log(LAMDA / (i + 1) + 1.0)) for i in range(NLAYERS)]
    I512 = np.eye(F, dtype=np.float32)
    WTh = np.stack([(1.0 - betas[i]) * I512 + betas[i] * conv_Ws[i]
                    for i in range(NLAYERS)]).astype(bf16)
    W1h = fc1_W.astype(bf16)
    B1h = fc1_b.reshape(1, F).astype(bf16)
    W2h = fc2_W.astype(bf16)
    B2h = fc2_b.reshape(1, NCLASS).astype(bf16)
    IDTh = np.eye(128, dtype=bf16)
    ONEh = np.ones((1, 128), dtype=bf16)

    md = np.arange(rl)
    pnew_diag = (md // csl) * (NC * csl) + (md % csl)  # + c*csl per core below

    in_maps = []
    for c in range(NC):
        r0 = c * rl
        ATf = adj[r0:r0 + rl, :].T[idx]          # [n, rl] permuted-row A^T slice
        ATf[pnew_diag + c * csl, md] += 1.0      # + I at permuted positions
        dloc = dinv[r0:r0 + rl]
        sc = np.empty((128, mt, 4), np.float32)
        sc[:, :, 0] = dloc.reshape(mt, 128).T
        sc[:, :, 1] = (1.0 / (9.0 * dloc)).reshape(mt, 128).T
        sc[:, :, 2] = (0.9 * dloc * dloc).reshape(mt, 128).T
        sc[:, :, 3] = (0.9 * dloc).reshape(mt, 128).T
        in_maps.append({
            "AT": ATf.astype(f8),
            "XT": np.ascontiguousarray(x[r0:r0 + rl, :].T).astype(bf16),
            "W1": W1h, "B1": B1h, "WT": WTh, "W2": W2h, "B2": B2h,
            "IDT": IDTh, "ONE": ONEh, "SC": sc,
        })
    return in_maps


def kernel(**inputs):
    global LAST_RESULTS, USED_FALLBACK
    x = np.asarray(inputs["x"], np.float32)
    adj = np.asarray(inputs["adj"], np.float32)
    fc1_W = np.asarray(inputs["fc1_W"], np.float32)
    fc1_b = np.asarray(inputs["fc1_b"], np.float32)
    conv_Ws = np.asarray(inputs["conv_Ws"], np.float32)
    fc2_W = np.asarray(inputs["fc2_W"], np.float32)
    fc2_b = np.asarray(inputs["fc2_b"], np.float32)
    try:
        in_maps = host_prep(x, adj, fc1_W, fc1_b, conv_Ws, fc2_W, fc2_b)
        from concourse.bass_utils import run_bass_kernel_spmd
        nc = build_nc()
        try:
            res = run_bass_kernel_spmd(nc, in_maps, core_ids=list(range(NC)))
        except ModuleNotFoundError:
            # BASS_TRACE set but this axon build lacks the NTFF hook module.
            os.environ["BASS_NEVER_TRACE"] = "1"
            res = run_bass_kernel_spmd(nc, in_maps, core_ids=list(range(NC)))
        LAST_RESULTS = res
        out = np.concatenate([np.asarray(res.results[c]["OUT"]) for c in range(NC)],
                             axis=0).astype(np.float32)
        return out
    except Exception:
        import traceback
        traceback.print_exc()
        USED_FALLBACK = True
        return _numpy_ref(x, adj, fc1_W, fc1_b, conv_Ws, fc2_W, fc2_b)


# revision 20
# speedup vs baseline: 31482.5056x; 2.6793x over previous
import os
import numpy as np

# nn_GCNII: H0 = relu(x@W1+b1); 8 layers of
#   M = 0.9*(P@H) + 0.1*H0 ; H = relu(M @ ((1-b)I + b*W_l)) ; out = -log_softmax(H@W2+b2)
# with P = D^-1/2 (A+I) D^-1/2 fixed.
#
# Device strategy (8 cores, row-sharded):
#  * Work in "tilde" space G = dinv * H. Then P@H = dinv * ((A+I) @ G) and the
#    per-layer update folds into per-partition scales:
#      pa  = (A+I)@G + I@B0c          (B0c = H0/(9*dinv), added via identity matmul)
#      Mt  = c2 * pa                  (c2 = 0.9*dinv^2; last layer uses c1 = 0.9*dinv)
#      G'  = relu(Mt @ Wt_l)          (Wt_l = (1-b)I + b*W_l)
#  * (A+I) entries are {0,1,2} -> exact in fp8; the 8MB/core slice stays SBUF-resident.
#  * Activations move in bf16 (1 cycle/row on PE vs 4 for fp32).
#  * Per-layer bf16 AllGather of G, chunked to overlap with compute.

N = 8192
NC = 8
RL = N // NC          # 1024 local rows
F = 512               # NFEAT == NHID
NCLASS = 64
NLAYERS = 8
LAMDA = 0.5
ALPHA = 0.1
NCH = 4               # allgather chunks per step

LAST_RESULTS = None
USED_FALLBACK = False


def _numpy_ref(x, adj, fc1_W, fc1_b, conv_Ws, fc2_W, fc2_b):
    n = adj.shape[0]
    A_hat = adj + np.eye(n, dtype=adj.dtype)
    dinv = 1.0 / np.sqrt(np.sum(A_hat, axis=0))
    P = dinv[:, None] * A_hat * dinv[None, :]
    H0 = np.maximum(x @ fc1_W + fc1_b, 0.0)
    H = H0
    for i in range(NLAYERS):
        beta = float(np.log(LAMDA / (i + 1) + 1.0))
        init_res = (1.0 - ALPHA) * (P @ H) + ALPHA * H0
        H = np.maximum((1.0 - beta) * init_res + beta * (init_res @ conv_Ws[i]), 0.0)
    logits = H @ fc2_W + fc2_b
    m = logits.max(axis=1, keepdims=True)
    lse = m + np.log(np.exp(logits - m).sum(axis=1, keepdims=True))
    return -(logits - lse)


def build_nc(n=N, nch=NCH, do_ag=True, do_w=True, do_reload=True, split_reload=4,
             alt_q=True, psa_bufs=2):
    import concourse.bacc as bacc
    import concourse.mybir as mybir
    from concourse import tile

    f32 = mybir.dt.float32
    bf16 = mybir.dt.bfloat16
    f8 = mybir.dt.float8e4

    rl = n // NC
    mt = rl // 128        # local row tiles
    kt = n // 128         # contraction tiles
    jt = F // 128         # feature tiles
    mtch = mt // nch      # m-tiles per AG chunk
    ktch = kt // nch      # k-tiles per chunk
    csl = rl // nch       # local rows per chunk

    # Bacc (not Bass): its finalize() runs generate_event_semaphores, which
    # splits multi-semaphore waits — walrus codegen allows only 1 sync wait
    # per instruction on TRN2 ("Too many sync wait commands" otherwise).
    nc = bacc.Bacc(target_bir_lowering=False, num_devices=NC)

    AT = nc.dram_tensor("AT", [n, rl], f8, kind="ExternalInput")
    XT = nc.dram_tensor("XT", [F, rl], bf16, kind="ExternalInput")
    W1 = nc.dram_tensor("W1", [F, F], bf16, kind="ExternalInput")
    B1 = nc.dram_tensor("B1", [1, F], bf16, kind="ExternalInput")
    WT = nc.dram_tensor("WT", [NLAYERS, F, F], bf16, kind="ExternalInput")
    W2 = nc.dram_tensor("W2", [F, NCLASS], bf16, kind="ExternalInput")
    B2 = nc.dram_tensor("B2", [1, NCLASS], bf16, kind="ExternalInput")
    IDT = nc.dram_tensor("IDT", [128, 128], bf16, kind="ExternalInput")
    ONE = nc.dram_tensor("ONE", [1, 128], bf16, kind="ExternalInput")
    SC = nc.dram_tensor("SC", [128, mt, 4], f32, kind="ExternalInput")
    OUT = nc.dram_tensor("OUT", [rl, NCLASS], f32, kind="ExternalOutput")

    gin = [[nc.dram_tensor(f"gin{s}_{q}", [csl, F], bf16)
            for q in range(nch)] for s in range(NLAYERS)]
    gf = [[nc.dram_tensor(f"gf{s}_{q}", [NC * csl, F], bf16, addr_space="Shared")
           for q in range(nch)] for s in range(NLAYERS)]

    AF = mybir.ActivationFunctionType
    ALU = mybir.AluOpType
    groups = [list(range(NC))]

    with tile.TileContext(nc) as tc:
        with (
            tc.tile_pool(name="res", bufs=1) as res,
            tc.tile_pool(name="gres", bufs=1) as gres,
            tc.tile_pool(name="mpool", bufs=3) as mpool,
            tc.tile_pool(name="tpool", bufs=4) as tpool,
            tc.tile_pool(name="spool", bufs=2) as spool,
            tc.tile_pool(name="opool", bufs=2) as opool,
            tc.tile_pool(name="psA", bufs=psa_bufs, space="PSUM") as psA,
            tc.tile_pool(name="psB", bufs=2, space="PSUM") as psB,
            tc.tile_pool(name="psT", bufs=2, space="PSUM") as psT,
        ):
            ATsb = res.tile([128, kt, rl], f8)
            WTsb = res.tile([128, NLAYERS, jt, F], bf16)
            B0csb = res.tile([128, mt, F], bf16)
            XTsb = res.tile([128, jt, rl], bf16)
            HFsb = res.tile([128, mt, F], bf16)
            W1sb = res.tile([128, jt, F], bf16)
            W2sb = res.tile([128, jt, NCLASS], bf16)
            IDTsb = res.tile([128, 128], bf16)
            ONEsb = res.tile([1, 128], bf16)
            B1sb = res.tile([1, F], bf16)
            B2sb = res.tile([1, NCLASS], bf16)
            SCsb = res.tile([128, mt, 4], f32)

            Gsb = [gres.tile([128, ktch, F], bf16, tag=f"g{q}", name=f"Gsb{q}")
                   for q in range(nch)]

            nc.sync.dma_start(IDTsb[:], IDT[:, :])
            nc.sync.dma_start(ONEsb[:], ONE[:, :])
            nc.sync.dma_start(B1sb[:], B1[:, :])
            nc.sync.dma_start(B2sb[:], B2[:, :])
            nc.sync.dma_start(SCsb[:], SC[:, :, :])
            nc.sync.dma_start(W1sb[:], W1[:, :].rearrange("(j p) f -> p j f", p=128))
            nc.sync.dma_start(W2sb[:], W2[:, :].rearrange("(j p) f -> p j f", p=128))
            nc.sync.dma_start(XTsb[:], XT[:, :].rearrange("(j p) m -> p j m", p=128))
            for l in range(NLAYERS):
                nc.sync.dma_start(WTsb[:, l, :, :],
                                  WT[l, :, :].rearrange("(j p) f -> p j f", p=128))
            nblk = 8 if kt % 8 == 0 else 1
            for i, b in enumerate(range(0, kt, nblk)):
                eng = nc.scalar if i % 2 else nc.sync
                eng.dma_start(
                    ATsb[:, b:b + nblk, :],
                    AT[b * 128:(b + nblk) * 128, :].rearrange("(k p) m -> p k m", p=128))

            # ---- fc1: H0 = relu(x@W1 + b1); emit G0 = dinv*H0, B0c = H0/(9*dinv)
            sdma_i = 0
            with nc.named_scope("fc1"):
                for q in range(nch):
                    for mi in range(mtch):
                        m = q * mtch + mi
                        p0 = psA.tile([128, F], f32, tag="pa")
                        nc.tensor.matmul(p0[:], ONEsb[:], B1sb[:], start=True, stop=False)
                        for j in range(jt):
                            nc.tensor.matmul(p0[:], XTsb[:, j, m * 128:(m + 1) * 128],
                                             W1sb[:, j, :], start=False, stop=(j == jt - 1))
                        gnew = mpool.tile([128, F], bf16, tag="gn")
                        nc.scalar.activation(gnew[:], p0[:], AF.Relu,
                                             0.0, SCsb[:, m, 0:1])
                        nc.scalar.activation(B0csb[:, m, :], p0[:], AF.Relu,
                                             0.0, SCsb[:, m, 1:2])
                        eng = nc.scalar if sdma_i % 2 else nc.sync
                        sdma_i += 1
                        eng.dma_start(gin[0][q][mi * 128:(mi + 1) * 128, :], gnew[:])
                    if do_ag:
                        nc.gpsimd.collective_compute(
                            "AllGather", ALU.bypass, replica_groups=groups,
                            ins=[gin[0][q][:, :]], outs=[gf[0][q][:, :]])

            # ---- GCNII layers
            dma_i = 0
            for l in range(NLAYERS):
                with nc.named_scope(f"L{l}"):
                    for q in range(nch):
                        if not do_reload:
                            continue
                        gfv = gf[l][q][:, :].rearrange("(k p) f -> p k f", p=128)
                        s = split_reload if split_reload else ktch
                        for kk in range(0, ktch, s):
                            eng = nc.scalar if (alt_q and dma_i % 2) else nc.sync
                            eng.dma_start(Gsb[q][:, kk:kk + s, :], gfv[:, kk:kk + s, :])
                            dma_i += 1
                    gch = None
                    for m in range(mt):
                        q, mi = divmod(m, mtch)
                        pa = psA.tile([128, F], f32, tag="pa")
                        for cq in range(nch):
                            for kk in range(ktch):
                                ktile = cq * ktch + kk
                                nc.tensor.matmul(
                                    pa[:], ATsb[:, ktile, m * 128:(m + 1) * 128],
                                    Gsb[cq][:, kk, :], start=(ktile == 0), stop=False)
                        nc.tensor.matmul(pa[:], IDTsb[:], B0csb[:, m, :],
                                         start=False, stop=True)
                        msb = mpool.tile([128, F], bf16, tag="m")
                        sidx = 2 if l < NLAYERS - 1 else 3
                        nc.scalar.activation(msb[:], pa[:], AF.Copy,
                                             0.0, SCsb[:, m, sidx:sidx + 1])
                        if do_w:
                            pb = psB.tile([128, F], f32, tag="pb")
                            for j in range(jt):
                                ptr = psT.tile([128, 128], bf16, tag="tr")
                                nc.tensor.transpose(ptr[:], msb[:, j * 128:(j + 1) * 128],
                                                    IDTsb[:])
                                mtj = tpool.tile([128, 128], bf16, tag="mt")
                                nc.vector.tensor_copy(mtj[:], ptr[:])
                                nc.tensor.matmul(pb[:], mtj[:], WTsb[:, l, j, :],
                                                 start=(j == 0), stop=(j == jt - 1))
                            src = pb
                        else:
                            src = pa
                        if l < NLAYERS - 1:
                            gnew = mpool.tile([128, F], bf16, tag="gn")
                            nc.scalar.activation(gnew[:], src[:], AF.Relu)
                            eng = nc.scalar if sdma_i % 2 else nc.sync
                            sdma_i += 1
                            eng.dma_start(gin[l + 1][q][mi * 128:(mi + 1) * 128, :],
                                          gnew[:])
                            if mi == mtch - 1 and do_ag:
                                nc.gpsimd.collective_compute(
                                    "AllGather", ALU.bypass, replica_groups=groups,
                                    ins=[gin[l + 1][q][:, :]],
                                    outs=[gf[l + 1][q][:, :]])
                        else:
                            nc.scalar.activation(HFsb[:, m, :], src[:], AF.Relu)

            # ---- fc2 + -log_softmax
            with nc.named_scope("fc2"):
                for m in range(mt):
                    pl = psB.tile([128, NCLASS], f32, tag="pb")
                    nc.tensor.matmul(pl[:], ONEsb[:], B2sb[:], start=True, stop=False)
                    for j in range(jt):
                        ptr = psT.tile([128, 128], bf16, tag="tr")
                        nc.tensor.transpose(ptr[:], HFsb[:, m, j * 128:(j + 1) * 128],
                                            IDTsb[:])
                        htj = tpool.tile([128, 128], bf16, tag="mt")
                        nc.vector.tensor_copy(htj[:], ptr[:])
                        nc.tensor.matmul(pl[:], htj[:], W2sb[:, j, :],
                                         start=False, stop=(j == jt - 1))
                    nmax = spool.tile([128, 1], f32, tag="nm")
                    nc.vector.tensor_reduce(nmax[:], pl[:], op=ALU.max,
                                            axis=mybir.AxisListType.X, negate=True)
                    esb = opool.tile([128, NCLASS], f32, tag="e")
                    ssum = spool.tile([128, 1], f32, tag="ss")
                    nc.scalar.activation(esb[:], pl[:], AF.Exp, nmax[:, 0:1], 1.0,
                                         accum_out=ssum[:])
                    lse = spool.tile([128, 1], f32, tag="ls")
                    nc.scalar.activation(lse[:], ssum[:], AF.Ln)
                    s2 = spool.tile([128, 1], f32, tag="s2")
                    nc.vector.tensor_tensor(s2[:], lse[:], nmax[:], ALU.subtract)
                    osb = opool.tile([128, NCLASS], f32, tag="o")
                    nc.vector.tensor_scalar(osb[:], pl[:], s2[:, 0:1], -1.0,
                                            op0=ALU.subtract, op1=ALU.mult)
                    nc.sync.dma_start(OUT[m * 128:(m + 1) * 128, :], osb[:])
    nc.finalize()
    return nc


def host_prep(x, adj, fc1_W, fc1_b, conv_Ws, fc2_W, fc2_b, n=N, nch=NCH):
    import ml_dtypes
    bf16 = ml_dtypes.bfloat16
    f8 = ml_dtypes.float8_e4m3

    rl = n // NC
    mt = rl // 128
    csl = rl // nch

    colsum = adj.sum(axis=0, dtype=np.float32) + 1.0
    dinv = (1.0 / np.sqrt(colsum)).astype(np.float32)

    # AT row permutation: AG chunk q concatenates cores' chunk-q slices, so
    # gathered G row order is (q, c, j); A^T rows must match.
    idx = np.arange(n).reshape(NC, nch, csl).transpose(1, 0, 2).ravel()

    betas = [float(np.log(LAMDA / (i + 1) + 1.0)) for i in range(NLAYERS)]
    I512 = np.eye(F, dtype=np.float32)
    WTh = np.stack([(1.0 - betas[i]) * I512 + betas[i] * conv_Ws[i]
                    for i in range(NLAYERS)]).astype(bf16)
    W1h = fc1_W.astype(bf16)
    B1h = fc1_b.reshape(1, F).astype(bf16)
    W2h = fc2_W.astype(bf16)
    B2h = fc2_b.reshape(1, NCLASS).astype(bf16)
    IDTh = np.eye(128, dtype=bf16)
    ONEh = np.ones((1, 128), dtype=bf16)

    md = np.arange(rl)
    pnew_diag = (md // csl) * (NC * csl) + (md % csl)  # + c*csl per core below

    in_maps = []
    for c in range(NC):
        r0 = c * rl
        ATf = adj[r0:r0 + rl, :].T[idx]          # [n, rl] permuted-row A^T slice
        ATf[pnew_diag + c * csl, md] += 1.0      # + I at permuted positions
        dloc = dinv[r0:r0 + rl]
        sc = np.empty((128, mt, 4), np.float32)
        sc[:, :, 0] = dloc.reshape(mt, 128).T
        sc[:, :, 1] = (1.0 / (9.0 * dloc)).reshape(mt, 128).T
        sc[:, :, 2] = (0.9 * dloc * dloc).reshape(mt, 128).T
        sc[:, :, 3] = (0.9 * dloc).reshape(mt, 128).T
        in_maps.append({
            "AT": ATf.astype(f8),
            "XT": np.ascontiguousarray(x[r0:r0 + rl, :].T).astype(bf16),
            "W1": W1h, "B1": B1h, "WT": WTh, "W2": W2h, "B2": B2h,
            "IDT": IDTh, "ONE": ONEh, "SC": sc,
        })
    return in_maps


def kernel(**inputs):
    global LAST_RESULTS, USED_FALLBACK
    x = np.asarray(inputs["x"], np.float32)
    adj = np.asarray(inputs["adj"], np.float32)
    fc1_W = np.asarray(inputs["fc1_W"], np.float32)
    fc1_b = np.asarray(inputs["fc1_b"], np.float32)
    conv_Ws = np.asarray(inputs["conv_Ws"], np.float32)
    fc2_W = np.asarray(inputs["fc2_W"], np.float32)
    fc2_b = np.asarray(inputs["fc2_b"], np.float32)
    try:
        in_maps = host_prep(x, adj, fc1_W, fc1_b, conv_Ws, fc2_W, fc2_b)
        from concourse.bass_utils import run_bass_kernel_spmd
        nc = build_nc()
        try:
            res = run_bass_kernel_spmd(nc, in_maps, core_ids=list(range(NC)))
        except ModuleNotFoundError:
            # BASS_TRACE set but this axon build lacks the NTFF hook module.
            os.environ["BASS_NEVER_TRACE"] = "1"
            res = run_bass_kernel_spmd(nc, in_maps, core_ids=list(range(NC)))
        LAST_RESULTS = res
        out = np.concatenate([np.asarray(res.results[c]["OUT"]) for c in range(NC)],
                             axis=0).astype(np.float32)
        return out
    except Exception:
        import traceback
        traceback.print_exc()
        USED_FALLBACK = True
        return _numpy_ref(x, adj, fc1_W, fc1_b, conv_Ws, fc2_W, fc2_b)
